# revision 3
# baseline (speedup 1.0000x reference)
"""CayleyMengerValidator loss kernel for 8 TRN2 NeuronCores.

Full inputs -> data-parallel shard over batch (2 batches/core) -> per-core
Bass kernel: indirect-DMA gather of sampled simplices, centered-Gram
computation, rose/regularity/degeneracy epilogue, per-partition partial
sums -> host combines to the scalar loss.
"""

import numpy as np

from concourse import bacc, bass, mybir
import concourse.tile as tile
from concourse.bass_utils import run_bass_kernel_spmd

P = 128
B, O, K1, D = 16, 8192, 5, 64
S = 2048
NCORES = 8
BPC = B // NCORES            # batches per core
SPC = BPC * S                # samples per core
COLS = SPC // P              # sample columns per partition
SUB = 8                      # gather tiles per supertile
NST = COLS // SUB
ROW = K1 * D                 # 320 floats per simplex row
NPAIR = 10
PAIR_OFF = [0, 4, 7, 9]      # i-major offsets of pairs (i,j), i<j in a 10-vector
N_TOTAL = B * S
EPS = 1e-8
DEG_THRESH = (24.0 * 1e-8) ** 2   # vol < 1e-8  <=>  det(gram4) < (24e-8)^2

f32 = mybir.dt.float32
i32 = mybir.dt.int32
Alu = mybir.AluOpType
Act = mybir.ActivationFunctionType
X = mybir.AxisListType.X


def _emit_supertile(nc, vpool, RhoP, N2, V):
    """V: [P, SUB, 5, 64] gathered simplices. Writes rho pairs into
    RhoP[:, st_cols, :] and centered squared norms into N2[:, st_cols, :]."""
    TT = nc.vector.tensor_tensor

    # centroid*5 = sum of the 5 points
    t1 = vpool.tile([P, SUB, D], f32, tag="t1", name="t1")
    t2 = vpool.tile([P, SUB, D], f32, tag="t2", name="t2")
    t3 = vpool.tile([P, SUB, D], f32, tag="t3", name="t3")
    C = vpool.tile([P, SUB, D], f32, tag="C", name="C")
    TT(out=t1[:], in0=V[:, :, 0, :], in1=V[:, :, 1, :], op=Alu.add)
    TT(out=t2[:], in0=V[:, :, 2, :], in1=V[:, :, 3, :], op=Alu.add)
    TT(out=t3[:], in0=t1[:], in1=t2[:], op=Alu.add)
    TT(out=t1[:], in0=t3[:], in1=V[:, :, 4, :], op=Alu.add)
    nc.vector.tensor_scalar_mul(C[:], t1[:], 0.2)

    # radial vectors r_i = v_i - c
    R = vpool.tile([P, SUB, K1, D], f32, tag="R", name="R")
    TT(
        out=R[:],
        in0=V[:],
        in1=C[:].unsqueeze(2).to_broadcast([P, SUB, K1, D]),
        op=Alu.subtract,
    )

    # pair products r_i * r_j (i-major) and diag squares
    PR = vpool.tile([P, SUB, NPAIR, D], f32, tag="PR", name="PR")
    for i in range(K1 - 1):
        nj = K1 - 1 - i
        o = PAIR_OFF[i]
        TT(
            out=PR[:, :, o : o + nj, :],
            in0=R[:, :, i, :].unsqueeze(2).to_broadcast([P, SUB, nj, D]),
            in1=R[:, :, i + 1 : K1, :],
            op=Alu.mult,
        )
    SQ = vpool.tile([P, SUB, K1, D], f32, tag="SQ", name="SQ")
    nc.scalar.square(SQ[:], R[:])

    # segmented reductions over D
    nc.vector.tensor_reduce(out=RhoP, in_=PR[:], axis=X, op=Alu.add)
    nc.vector.tensor_reduce(out=N2, in_=SQ[:], axis=X, op=Alu.add)


def _emit_epilogue(nc, epool, RhoP, N2, SUMS):
    """RhoP: [P, COLS, 10] pair dots (centered), N2: [P, COLS, 5] sq norms.
    SUMS: [P, 3] = (sum cos_pairsum, sum regularity, sum degenerate)."""
    TT = nc.vector.tensor_tensor
    STT = nc.vector.scalar_tensor_tensor

    def tile3(k, name):
        return epool.tile([P, COLS, k], f32, tag=name, name=name)

    # --- pairwise squared distances d2_ij = n2_i + n2_j - 2 rho_ij
    H = tile3(NPAIR, "H")
    for i in range(K1 - 1):
        nj = K1 - 1 - i
        o = PAIR_OFF[i]
        TT(
            out=H[:, :, o : o + nj],
            in0=N2[:, :, i].unsqueeze(2).to_broadcast([P, COLS, nj]),
            in1=N2[:, :, i + 1 : K1],
            op=Alu.add,
        )
    D2 = tile3(NPAIR, "D2")
    STT(out=D2[:], in0=RhoP[:], scalar=-2.0, in1=H[:], op0=Alu.mult, op1=Alu.add)

    # --- edges and regularity = min_edge / max_edge
    E = tile3(NPAIR, "E")
    nc.vector.tensor_scalar_max(E[:], D2[:], 1e-12)
    nc.scalar.sqrt(E[:], E[:])
    EMIN = tile3(1, "EMIN")
    EMAX = tile3(1, "EMAX")
    nc.vector.tensor_reduce(out=EMIN[:], in_=E[:], axis=X, op=Alu.min)
    nc.vector.tensor_reduce(out=EMAX[:], in_=E[:], axis=X, op=Alu.max)
    nc.vector.tensor_scalar_max(EMAX[:], EMAX[:], EPS)
    REMAX = tile3(1, "REMAX")
    nc.vector.reciprocal(REMAX[:], EMAX[:])
    REG = tile3(1, "REG")
    TT(out=REG[:], in0=EMIN[:], in1=REMAX[:], op=Alu.mult)

    # --- cos_ij = rho_ij / (n_i n_j); sum over the 10 pairs
    N2C = tile3(K1, "N2C")
    nc.vector.tensor_scalar_max(N2C[:], N2[:], 1e-16)
    IN2 = tile3(K1, "IN2")
    nc.vector.reciprocal(IN2[:], N2C[:])
    IP = tile3(NPAIR, "IP")
    for i in range(K1 - 1):
        nj = K1 - 1 - i
        o = PAIR_OFF[i]
        TT(
            out=IP[:, :, o : o + nj],
            in0=IN2[:, :, i].unsqueeze(2).to_broadcast([P, COLS, nj]),
            in1=IN2[:, :, i + 1 : K1],
            op=Alu.mult,
        )
    nc.scalar.sqrt(IP[:], IP[:])
    COS = tile3(NPAIR, "COS")
    TT(out=COS[:], in0=RhoP[:], in1=IP[:], op=Alu.mult)
    CS = tile3(1, "CS")
    nc.vector.tensor_reduce(out=CS[:], in_=COS[:], axis=X, op=Alu.add)

    # --- Cayley-Menger degeneracy via det of the 4x4 gram of w_p = v_p - v_0
    # g_pp = n2_p + n2_0 - 2 rho_0p           (p = 1..4)
    # g_pq = rho_pq - rho_0p - rho_0q + n2_0  (1 <= p < q <= 4)
    Gd = tile3(4, "Gd")
    TT(
        out=Gd[:],
        in0=N2[:, :, 1:K1],
        in1=N2[:, :, 0].unsqueeze(2).to_broadcast([P, COLS, 4]),
        op=Alu.add,
    )
    STT(out=Gd[:], in0=RhoP[:, :, 0:4], scalar=-2.0, in1=Gd[:], op0=Alu.mult, op1=Alu.add)
    Go = tile3(6, "Go")
    TT(
        out=Go[:],
        in0=RhoP[:, :, 4:10],
        in1=N2[:, :, 0].unsqueeze(2).to_broadcast([P, COLS, 6]),
        op=Alu.add,
    )
    # subtract rho_0p (p-major: p=1 covers 3 pairs, p=2 covers 2, p=3 covers 1)
    po = [(0, 0, 3), (3, 1, 2), (5, 2, 1)]
    for o, pidx, n in po:
        TT(
            out=Go[:, :, o : o + n],
            in0=Go[:, :, o : o + n],
            in1=RhoP[:, :, pidx].unsqueeze(2).to_broadcast([P, COLS, n]),
            op=Alu.subtract,
        )
    # subtract rho_0q (q lists are contiguous slices of RhoP[:, :, 0:4])
    qo = [(0, 1, 3), (3, 2, 2), (5, 3, 1)]
    for o, q0, n in qo:
        TT(
            out=Go[:, :, o : o + n],
            in0=Go[:, :, o : o + n],
            in1=RhoP[:, :, q0 : q0 + n],
            op=Alu.subtract,
        )

    # det via Laplace on the first two rows: 12 2x2 minors
    # matrix [[A,B,C,D],[B,E,F,G],[C,F,H,I],[D,G,I,J]]
    A = Gd[:, :, 0]
    Ev = Gd[:, :, 1]
    Hv = Gd[:, :, 2]
    J = Gd[:, :, 3]
    Bv = Go[:, :, 0]
    Cv = Go[:, :, 1]
    Dv = Go[:, :, 2]
    F = Go[:, :, 3]
    G = Go[:, :, 4]
    I = Go[:, :, 5]
    svals = [(A, Ev, Bv, Bv), (A, F, Cv, Bv), (A, G, Dv, Bv),
             (Bv, F, Cv, Ev), (Bv, G, Dv, Ev), (Cv, G, Dv, F)]
    cvals = [(Hv, J, I, I), (F, J, I, G), (F, I, Hv, G),
             (Cv, J, I, Dv), (Cv, I, Hv, Dv), (Cv, G, F, Dv)]
    SV = tile3(6, "SV")
    CV = tile3(6, "CV")
    TMP = tile3(1, "TMP")
    for dst, vals in ((SV, svals), (CV, cvals)):
        for k, (x1, y1, x2, y2) in enumerate(vals):
            d = dst[:, :, k]
            TT(out=d, in0=x1, in1=y1, op=Alu.mult)
            TT(out=TMP[:, :, 0], in0=x2, in1=y2, op=Alu.mult)
            TT(out=d, in0=d, in1=TMP[:, :, 0], op=Alu.subtract)
    PV = tile3(6, "PV")
    TT(out=PV[:], in0=SV[:], in1=CV[:], op=Alu.mult)
    # det = p0 - p1 + p2 + p3 - p4 + p5
    DET = tile3(1, "DET")
    NEG = tile3(1, "NEG")
    TT(out=DET[:, :, 0], in0=PV[:, :, 0], in1=PV[:, :, 2], op=Alu.add)
    TT(out=DET[:, :, 0], in0=DET[:, :, 0], in1=PV[:, :, 3], op=Alu.add)
    TT(out=DET[:, :, 0], in0=DET[:, :, 0], in1=PV[:, :, 5], op=Alu.add)
    TT(out=NEG[:, :, 0], in0=PV[:, :, 1], in1=PV[:, :, 4], op=Alu.add)
    TT(out=DET[:, :, 0], in0=DET[:, :, 0], in1=NEG[:, :, 0], op=Alu.subtract)
    DEG = tile3(1, "DEG")
    nc.vector.tensor_scalar(DEG[:], DET[:], DEG_THRESH, None, op0=Alu.is_lt)

    # --- per-partition partial sums
    nc.vector.tensor_reduce(out=SUMS[:, 0:1], in_=CS[:, :, 0], axis=X, op=Alu.add)
    nc.vector.tensor_reduce(out=SUMS[:, 1:2], in_=REG[:, :, 0], axis=X, op=Alu.add)
    nc.vector.tensor_reduce(out=SUMS[:, 2:3], in_=DEG[:, :, 0], axis=X, op=Alu.add)


def build():
    nc = bacc.Bacc(
        "TRN2",
        target_bir_lowering=False,
        debug=False,
        enable_asserts=False,
        num_devices=NCORES,
    )
    pred = nc.dram_tensor("pred", [BPC * O, ROW], f32, kind="ExternalInput").ap()
    idx = nc.dram_tensor("idx", [P, COLS], i32, kind="ExternalInput").ap()
    out = nc.dram_tensor("out", [P, 3], f32, kind="ExternalOutput").ap()

    with tile.TileContext(nc) as tc:
        with (
            tc.tile_pool(name="const", bufs=1) as cpool,
            tc.tile_pool(name="v", bufs=3) as vpool,
            tc.tile_pool(name="stat", bufs=1) as spool,
        ):
            idx_sb = cpool.tile([P, COLS], i32)
            nc.sync.dma_start(out=idx_sb[:], in_=idx)

            RhoP = spool.tile([P, COLS, NPAIR], f32)
            N2 = spool.tile([P, COLS, K1], f32)

            for st in range(NST):
                V = vpool.tile([P, SUB, K1, D], f32, tag="V", name="V")
                for s in range(SUB):
                    c = st * SUB + s
                    nc.gpsimd.indirect_dma_start(
                        out=V[:, s].opt(),
                        out_offset=None,
                        in_=pred,
                        in_offset=bass.IndirectOffsetOnAxis(
                            ap=idx_sb[:, c : c + 1], axis=0
                        ),
                    )
                cs = slice(st * SUB, (st + 1) * SUB)
                _emit_supertile(nc, vpool, RhoP[:, cs, :], N2[:, cs, :], V)

            SUMS = spool.tile([P, 3], f32)
            _emit_epilogue(nc, spool, RhoP[:], N2[:], SUMS)
            nc.sync.dma_start(out=out, in_=SUMS[:])

    nc.compile()
    return nc


_NC = None


def _get_nc():
    global _NC
    if _NC is None:
        _NC = build()
    return _NC


def make_in_maps(predicted_simplices, sample_indices):
    pred = np.ascontiguousarray(predicted_simplices, dtype=np.float32)
    idx = np.ascontiguousarray(sample_indices, dtype=np.int32)
    in_maps = []
    for c in range(NCORES):
        p = pred[c * BPC : (c + 1) * BPC].reshape(BPC * O, ROW)
        # global (batch, sample) index -> local flat row id in this core's shard
        ix = idx[c * BPC : (c + 1) * BPC] + (np.arange(BPC, dtype=np.int32) * O)[:, None]
        ix = ix.reshape(P, COLS)
        in_maps.append(
            {"pred": np.ascontiguousarray(p), "idx": np.ascontiguousarray(ix)}
        )
    return in_maps


def combine(results):
    cs_total = 0.0
    reg_total = 0.0
    deg_total = 0.0
    for r in results:
        o = r["out"].astype(np.float64)
        cs_total += o[:, 0].sum()
        reg_total += o[:, 1].sum()
        deg_total += o[:, 2].sum()
    n = float(N_TOTAL)
    rose_loss = 0.5 - cs_total / (20.0 * n)
    quality_loss = 1.0 - reg_total / n
    volume_loss = deg_total / n
    total = 0.5 * rose_loss + 0.3 * quality_loss + 0.2 * volume_loss
    return np.float32(total)


def kernel(predicted_simplices, sample_indices):
    nc = _get_nc()
    in_maps = make_in_maps(predicted_simplices, sample_indices)
    res = run_bass_kernel_spmd(nc, in_maps, core_ids=list(range(NCORES)))
    return combine(res.results)


# revision 6
# speedup vs baseline: 1.1102x; 1.1102x over previous
"""CayleyMengerValidator loss kernel for 8 TRN2 NeuronCores.

Full inputs -> data-parallel shard over batch (2 batches/core) -> per-core
Bass kernel: dma_gather of sampled simplices, raw-Gram computation (bf16
products, tree + segmented reduce), epilogue derives rose/regularity/
Cayley-Menger degeneracy per sample, per-partition partial sums -> host
combines to the scalar loss.
"""

import numpy as np

from concourse import bacc, bass, mybir
import concourse.tile as tile
from concourse.bass_utils import run_bass_kernel_spmd

P = 128
B, O, K1, D = 16, 8192, 5, 64
S = 2048
NCORES = 8
BPC = B // NCORES            # batches per core
SPC = BPC * S                # samples per core
COLS = SPC // P              # sample columns per partition
SUB = 8                      # gather tiles per supertile
NST = COLS // SUB
ROW = K1 * D                 # 320 floats per simplex row
NPAIR = 10
PAIR_OFF = [0, 4, 7, 9]      # i-major offsets of pairs (i,j), i<j in a 10-vector
N_TOTAL = B * S
EPS = 1e-8
DEG_THRESH = (24.0 * 1e-8) ** 2   # vol < 1e-8  <=>  det(gram4) < (24e-8)^2

f32 = mybir.dt.float32
bf16 = mybir.dt.bfloat16
i16 = mybir.dt.int16
Alu = mybir.AluOpType
Act = mybir.ActivationFunctionType
X = mybir.AxisListType.X
XY = mybir.AxisListType.XY


def _emit_supertile(nc, vpool, pred, Gst, Nst, idx_ap):
    """Gather SUB*P simplices and write raw pair dots into Gst [P,SUB,10]
    and raw squared norms into Nst [P,SUB,5] (both f32 views)."""
    TT = nc.vector.tensor_tensor

    V = vpool.tile([P, SUB, K1, D], f32, tag="V", name="V")
    nc.gpsimd.dma_gather(
        out_ap=V[:].rearrange("p s k d -> p s (k d)"),
        in_ap=pred,
        idxs_ap=idx_ap,
        num_idxs=SUB * P,
        num_idxs_reg=SUB * P,
        elem_size=ROW,
    )

    VB = vpool.tile([P, SUB, K1, D], bf16, tag="VB", name="VB")
    nc.scalar.copy(VB[:], V[:])
    SQ = vpool.tile([P, SUB, K1, D], bf16, tag="SQ", name="SQ")
    nc.scalar.square(SQ[:], V[:])

    PR = vpool.tile([P, SUB, NPAIR, D], bf16, tag="PR", name="PR")
    for i in range(K1 - 1):
        nj = K1 - 1 - i
        o = PAIR_OFF[i]
        TT(
            out=PR[:, :, o : o + nj, :],
            in0=VB[:, :, i, :].unsqueeze(2).to_broadcast([P, SUB, nj, D]),
            in1=VB[:, :, i + 1 : K1, :],
            op=Alu.mult,
        )

    # two bf16 tree levels then f32 segmented reduce
    PR1 = vpool.tile([P, SUB, NPAIR, D // 2], bf16, tag="PR1", name="PR1")
    TT(out=PR1[:], in0=PR[:, :, :, 0 : D // 2], in1=PR[:, :, :, D // 2 : D], op=Alu.add)
    PR2 = vpool.tile([P, SUB, NPAIR, D // 4], bf16, tag="PR2", name="PR2")
    TT(out=PR2[:], in0=PR1[:, :, :, 0 : D // 4], in1=PR1[:, :, :, D // 4 :], op=Alu.add)
    nc.vector.tensor_reduce(out=Gst, in_=PR2[:], axis=X, op=Alu.add)

    SQ1 = vpool.tile([P, SUB, K1, D // 2], bf16, tag="SQ1", name="SQ1")
    TT(out=SQ1[:], in0=SQ[:, :, :, 0 : D // 2], in1=SQ[:, :, :, D // 2 : D], op=Alu.add)
    SQ2 = vpool.tile([P, SUB, K1, D // 4], bf16, tag="SQ2", name="SQ2")
    TT(out=SQ2[:], in0=SQ1[:, :, :, 0 : D // 4], in1=SQ1[:, :, :, D // 4 :], op=Alu.add)
    nc.vector.tensor_reduce(out=Nst, in_=SQ2[:], axis=X, op=Alu.add)


def _emit_epilogue(nc, epool, G, N, SUMS):
    """G: [P, COLS, 10] raw pair dots, N: [P, COLS, 5] raw squared norms.
    SUMS: [P, 3] = (sum cos_pairsum, sum regularity, sum degenerate)."""
    TT = nc.vector.tensor_tensor
    STT = nc.vector.scalar_tensor_tensor
    CP = nc.vector.tensor_copy

    def tile3(k, name):
        return epool.tile([P, COLS, k], f32, tag=name, name=name)

    def pair_combine(dst, src, op):
        # dst[pair(i,j)] = src_i (op) src_j over the 10 i-major pairs
        for i in range(K1 - 1):
            nj = K1 - 1 - i
            o = PAIR_OFF[i]
            TT(
                out=dst[:, :, o : o + nj],
                in0=src[:, :, i].unsqueeze(2).to_broadcast([P, COLS, nj]),
                in1=src[:, :, i + 1 : K1],
                op=op,
            )

    # --- full 5x5 gram matrix (for row sums) -> centering corrections
    GF = tile3(25, "GF")
    GF5 = GF[:].rearrange("p c (a b) -> p c a b", a=5)
    CP(GF[:, :, 0:25:6], N[:])                      # diagonal
    for i in range(K1 - 1):
        nj = K1 - 1 - i
        o = PAIR_OFF[i]
        CP(GF5[:, :, i, i + 1 : K1], G[:, :, o : o + nj])       # upper row i
        CP(GF5[:, :, i + 1 : K1, i], G[:, :, o : o + nj])       # lower col i
    M = tile3(K1, "M")                               # row sums = 5 * v_i . c
    nc.vector.tensor_reduce(out=M[:], in_=GF5, axis=X, op=Alu.add)
    Q = tile3(1, "Q")                                # sum of all = 25 * c . c
    nc.vector.tensor_reduce(out=Q[:, :, 0], in_=GF5, axis=XY, op=Alu.add)

    # centered pair dots / squared norms (only the cos path needs these)
    MP = tile3(NPAIR, "MP")
    pair_combine(MP, M, Alu.add)
    RC = tile3(NPAIR, "RC")
    STT(out=RC[:], in0=MP[:], scalar=-0.2, in1=G[:], op0=Alu.mult, op1=Alu.add)
    STT(
        out=RC[:],
        in0=Q[:].to_broadcast([P, COLS, NPAIR]),
        scalar=1.0 / 25.0,
        in1=RC[:],
        op0=Alu.mult,
        op1=Alu.add,
    )
    NC = tile3(K1, "NC")
    STT(out=NC[:], in0=M[:], scalar=-0.4, in1=N[:], op0=Alu.mult, op1=Alu.add)
    STT(
        out=NC[:],
        in0=Q[:].to_broadcast([P, COLS, K1]),
        scalar=1.0 / 25.0,
        in1=NC[:],
        op0=Alu.mult,
        op1=Alu.add,
    )

    RES = epool.tile([P, 3, COLS], f32, tag="RES", name="RES")

    # --- pairwise squared distances (raw values: centroid cancels)
    H = tile3(NPAIR, "H")
    pair_combine(H, N, Alu.add)
    D2 = tile3(NPAIR, "D2")
    STT(out=D2[:], in0=G[:], scalar=-2.0, in1=H[:], op0=Alu.mult, op1=Alu.add)
    E = tile3(NPAIR, "E")
    BIAS12 = epool.tile([P, 1], f32, tag="BIAS12", name="BIAS12")
    nc.vector.memset(BIAS12[:], 1e-12)
    nc.scalar.activation(E[:], D2[:], Act.Sqrt, bias=BIAS12[:])   # clips tiny negatives
    EMIN = tile3(1, "EMIN")
    EMAX = tile3(1, "EMAX")
    nc.vector.tensor_reduce(out=EMIN[:], in_=E[:], axis=X, op=Alu.min)
    nc.vector.tensor_reduce(out=EMAX[:], in_=E[:], axis=X, op=Alu.max)
    REMAX = tile3(1, "REMAX")
    nc.vector.reciprocal(REMAX[:], EMAX[:])
    TT(out=RES[:, 1, :], in0=EMIN[:, :, 0], in1=REMAX[:, :, 0], op=Alu.mult)

    # --- cos_ij = rho_c_ij / (n_i n_j); sum over the 10 pairs
    IN2 = tile3(K1, "IN2")
    nc.vector.reciprocal(IN2[:], NC[:])
    IP = tile3(NPAIR, "IP")
    pair_combine(IP, IN2, Alu.mult)
    nc.scalar.sqrt(IP[:], IP[:])
    COS = tile3(NPAIR, "COS")
    TT(out=COS[:], in0=RC[:], in1=IP[:], op=Alu.mult)
    nc.vector.tensor_reduce(out=RES[:, 0, :], in_=COS[:], axis=X, op=Alu.add)

    # --- Cayley-Menger degeneracy via det of the 4x4 gram of w_p = v_p - v_0
    # (raw values: w_p . w_q = G_pq - G_0p - G_0q + n2_0)
    Gd = tile3(4, "Gd")
    TT(
        out=Gd[:],
        in0=N[:, :, 1:K1],
        in1=N[:, :, 0].unsqueeze(2).to_broadcast([P, COLS, 4]),
        op=Alu.add,
    )
    STT(out=Gd[:], in0=G[:, :, 0:4], scalar=-2.0, in1=Gd[:], op0=Alu.mult, op1=Alu.add)
    Go = tile3(6, "Go")
    TT(
        out=Go[:],
        in0=G[:, :, 4:10],
        in1=N[:, :, 0].unsqueeze(2).to_broadcast([P, COLS, 6]),
        op=Alu.add,
    )
    # subtract G_0p (p-major: p=1 covers 3 pairs, p=2 covers 2, p=3 covers 1)
    for o, pidx, n in [(0, 0, 3), (3, 1, 2), (5, 2, 1)]:
        TT(
            out=Go[:, :, o : o + n],
            in0=Go[:, :, o : o + n],
            in1=G[:, :, pidx].unsqueeze(2).to_broadcast([P, COLS, n]),
            op=Alu.subtract,
        )
    # subtract G_0q (q lists are contiguous slices of G[:, :, 0:4])
    for o, q0, n in [(0, 1, 3), (3, 2, 2), (5, 3, 1)]:
        TT(
            out=Go[:, :, o : o + n],
            in0=Go[:, :, o : o + n],
            in1=G[:, :, q0 : q0 + n],
            op=Alu.subtract,
        )

    # det via Laplace on the first two rows: 12 2x2 minors
    # matrix [[A,B,C,D],[B,E,F,G],[C,F,H,I],[D,G,I,J]]
    A = Gd[:, :, 0]
    Ev = Gd[:, :, 1]
    Hv = Gd[:, :, 2]
    J = Gd[:, :, 3]
    Bv = Go[:, :, 0]
    Cv = Go[:, :, 1]
    Dv = Go[:, :, 2]
    F = Go[:, :, 3]
    G4 = Go[:, :, 4]
    I4 = Go[:, :, 5]
    svals = [(A, Ev, Bv, Bv), (A, F, Cv, Bv), (A, G4, Dv, Bv),
             (Bv, F, Cv, Ev), (Bv, G4, Dv, Ev), (Cv, G4, Dv, F)]
    cvals = [(Hv, J, I4, I4), (F, J, I4, G4), (F, I4, Hv, G4),
             (Cv, J, I4, Dv), (Cv, I4, Hv, Dv), (Cv, G4, F, Dv)]
    SV = tile3(6, "SV")
    CV = tile3(6, "CV")
    TMP = tile3(1, "TMP")
    for dst, vals in ((SV, svals), (CV, cvals)):
        for k, (x1, y1, x2, y2) in enumerate(vals):
            d = dst[:, :, k]
            TT(out=d, in0=x1, in1=y1, op=Alu.mult)
            TT(out=TMP[:, :, 0], in0=x2, in1=y2, op=Alu.mult)
            TT(out=d, in0=d, in1=TMP[:, :, 0], op=Alu.subtract)
    PV = tile3(6, "PV")
    TT(out=PV[:], in0=SV[:], in1=CV[:], op=Alu.mult)
    # det = p0 - p1 + p2 + p3 - p4 + p5
    DET = tile3(1, "DET")
    NEG = tile3(1, "NEG")
    TT(out=DET[:, :, 0], in0=PV[:, :, 0], in1=PV[:, :, 2], op=Alu.add)
    TT(out=DET[:, :, 0], in0=DET[:, :, 0], in1=PV[:, :, 3], op=Alu.add)
    TT(out=DET[:, :, 0], in0=DET[:, :, 0], in1=PV[:, :, 5], op=Alu.add)
    TT(out=NEG[:, :, 0], in0=PV[:, :, 1], in1=PV[:, :, 4], op=Alu.add)
    TT(out=DET[:, :, 0], in0=DET[:, :, 0], in1=NEG[:, :, 0], op=Alu.subtract)
    nc.vector.tensor_scalar(RES[:, 2, :], DET[:, :, 0], DEG_THRESH, None, op0=Alu.is_lt)

    # --- per-partition partial sums
    nc.vector.tensor_reduce(out=SUMS[:], in_=RES[:], axis=X, op=Alu.add)


def build():
    nc = bacc.Bacc(
        "TRN2",
        target_bir_lowering=False,
        debug=False,
        enable_asserts=False,
        num_devices=NCORES,
    )
    pred = nc.dram_tensor("pred", [BPC * O, ROW], f32, kind="ExternalInput").ap()
    idx = nc.dram_tensor("idx", [P, SPC // 16], i16, kind="ExternalInput").ap()
    out = nc.dram_tensor("out", [P, 3], f32, kind="ExternalOutput").ap()

    with tile.TileContext(nc) as tc:
        with (
            tc.tile_pool(name="const", bufs=1) as cpool,
            tc.tile_pool(name="v", bufs=3) as vpool,
            tc.tile_pool(name="stat", bufs=1) as spool,
        ):
            idx_sb = cpool.tile([P, SPC // 16], i16)
            nc.sync.dma_start(out=idx_sb[:], in_=idx)

            G = spool.tile([P, COLS, NPAIR], f32)
            N = spool.tile([P, COLS, K1], f32)

            ipc = SUB * P // 16          # idx columns per supertile
            for st in range(NST):
                cs = slice(st * SUB, (st + 1) * SUB)
                _emit_supertile(
                    nc,
                    vpool,
                    pred,
                    G[:, cs, :],
                    N[:, cs, :],
                    idx_sb[:, st * ipc : (st + 1) * ipc],
                )

            SUMS = spool.tile([P, 3], f32)
            _emit_epilogue(nc, spool, G[:], N[:], SUMS)
            nc.sync.dma_start(out=out, in_=SUMS[:])

    nc.compile()
    return nc


_NC = None


def _get_nc():
    global _NC
    if _NC is None:
        _NC = build()
    return _NC


def make_in_maps(predicted_simplices, sample_indices):
    pred = np.ascontiguousarray(predicted_simplices, dtype=np.float32)
    idx = np.ascontiguousarray(sample_indices, dtype=np.int32)
    in_maps = []
    for c in range(NCORES):
        p = pred[c * BPC : (c + 1) * BPC].reshape(BPC * O, ROW)
        # global (batch, sample) index -> local flat row id in this core's shard
        rowids = (
            idx[c * BPC : (c + 1) * BPC]
            + (np.arange(BPC, dtype=np.int32) * O)[:, None]
        ).reshape(SPC)
        # dma_gather index layout: flat position g lives at [g % 16, g // 16]
        ix = np.zeros((P, SPC // 16), np.int16)
        ix[:16] = rowids.astype(np.int16).reshape(SPC // 16, 16).T
        in_maps.append(
            {"pred": np.ascontiguousarray(p), "idx": np.ascontiguousarray(ix)}
        )
    return in_maps


def combine(results):
    cs_total = 0.0
    reg_total = 0.0
    deg_total = 0.0
    for r in results:
        o = r["out"].astype(np.float64)
        cs_total += o[:, 0].sum()
        reg_total += o[:, 1].sum()
        deg_total += o[:, 2].sum()
    n = float(N_TOTAL)
    rose_loss = 0.5 - cs_total / (20.0 * n)
    quality_loss = 1.0 - reg_total / n
    volume_loss = deg_total / n
    total = 0.5 * rose_loss + 0.3 * quality_loss + 0.2 * volume_loss
    return np.float32(total)


def kernel(predicted_simplices, sample_indices):
    nc = _get_nc()
    in_maps = make_in_maps(predicted_simplices, sample_indices)
    res = run_bass_kernel_spmd(nc, in_maps, core_ids=list(range(NCORES)))
    return combine(res.results)


# revision 9
# speedup vs baseline: 1.3210x; 1.1898x over previous
"""CayleyMengerValidator loss kernel for 8 TRN2 NeuronCores.

Full inputs -> data-parallel shard over batch (2 batches/core) -> per-core
Bass kernel: dma_gather of sampled simplices, raw-Gram computation (bf16
products, tree + segmented reduce), epilogue derives rose/regularity/
Cayley-Menger degeneracy per sample, per-partition partial sums -> host
combines to the scalar loss.
"""

import numpy as np

from concourse import bacc, bass, mybir
import concourse.tile as tile
from concourse.bass_utils import run_bass_kernel_spmd

P = 128
B, O, K1, D = 16, 8192, 5, 64
S = 2048
NCORES = 8
BPC = B // NCORES            # batches per core
SPC = BPC * S                # samples per core
COLS = SPC // P              # sample columns per partition
SUB = 8                      # gather tiles per supertile
NST = COLS // SUB
ROW = K1 * D                 # 320 floats per simplex row
NPAIR = 10
PAIR_OFF = [0, 4, 7, 9]      # i-major offsets of pairs (i,j), i<j in a 10-vector
N_TOTAL = B * S
EPS = 1e-8
DEG_THRESH = (24.0 * 1e-8) ** 2   # vol < 1e-8  <=>  det(gram4) < (24e-8)^2
DEBUG_DUMP = False

f32 = mybir.dt.float32
bf16 = mybir.dt.bfloat16
i16 = mybir.dt.int16
Alu = mybir.AluOpType
Act = mybir.ActivationFunctionType
X = mybir.AxisListType.X
XY = mybir.AxisListType.XY


def _emit_supertile(nc, vpool, pred, Gst, Nst, idx_ap, st):
    """Gather SUB*P simplices and write raw pair dots into Gst [P,SUB,10]
    and raw squared norms into Nst [P,SUB,5] (both f32 views)."""
    TT = nc.vector.tensor_tensor

    V = vpool.tile([P, SUB, K1, D], f32, tag="V", name="V")
    nc.gpsimd.dma_gather(
        out_ap=V[:].rearrange("p s k d -> p s (k d)"),
        in_ap=pred,
        idxs_ap=idx_ap,
        num_idxs=SUB * P,
        num_idxs_reg=SUB * P,
        elem_size=ROW,
        single_packet=False,
        queue_num=st,
    )

    VB = vpool.tile([P, SUB, K1, D], bf16, tag="VB", name="VB")
    nc.scalar.copy(VB[:], V[:])
    SQ = vpool.tile([P, SUB, K1, D], bf16, tag="SQ", name="SQ")
    nc.scalar.square(SQ[:], V[:])

    PR = vpool.tile([P, SUB, NPAIR, D], bf16, tag="PR", name="PR")
    for i in range(K1 - 1):
        nj = K1 - 1 - i
        o = PAIR_OFF[i]
        TT(
            out=PR[:, :, o : o + nj, :],
            in0=VB[:, :, i, :].unsqueeze(2).to_broadcast([P, SUB, nj, D]),
            in1=VB[:, :, i + 1 : K1, :],
            op=Alu.mult,
        )

    # two bf16 tree levels then f32 segmented reduce
    PR1 = vpool.tile([P, SUB, NPAIR, D // 2], bf16, tag="PR1", name="PR1")
    TT(out=PR1[:], in0=PR[:, :, :, 0 : D // 2], in1=PR[:, :, :, D // 2 : D], op=Alu.add)
    PR2 = vpool.tile([P, SUB, NPAIR, D // 4], bf16, tag="PR2", name="PR2")
    TT(out=PR2[:], in0=PR1[:, :, :, 0 : D // 4], in1=PR1[:, :, :, D // 4 :], op=Alu.add)
    nc.vector.tensor_reduce(out=Gst, in_=PR2[:], axis=X, op=Alu.add)

    SQ1 = vpool.tile([P, SUB, K1, D // 2], bf16, tag="SQ1", name="SQ1")
    TT(out=SQ1[:], in0=SQ[:, :, :, 0 : D // 2], in1=SQ[:, :, :, D // 2 : D], op=Alu.add)
    SQ2 = vpool.tile([P, SUB, K1, D // 4], bf16, tag="SQ2", name="SQ2")
    TT(out=SQ2[:], in0=SQ1[:, :, :, 0 : D // 4], in1=SQ1[:, :, :, D // 4 :], op=Alu.add)
    nc.vector.tensor_reduce(out=Nst, in_=SQ2[:], axis=X, op=Alu.add)


def _emit_epilogue(nc, epool, G, N, SUMS):
    """G: [P, COLS, 10] raw pair dots, N: [P, COLS, 5] raw squared norms.
    SUMS: [P, 3] = (sum cos_pairsum, sum regularity, sum degenerate)."""
    TT = nc.vector.tensor_tensor
    STT = nc.vector.scalar_tensor_tensor
    CP = nc.vector.tensor_copy

    def tile3(k, name):
        return epool.tile([P, COLS, k], f32, tag=name, name=name)

    def pair_combine(dst, src, op):
        # dst[pair(i,j)] = src_i (op) src_j over the 10 i-major pairs
        for i in range(K1 - 1):
            nj = K1 - 1 - i
            o = PAIR_OFF[i]
            TT(
                out=dst[:, :, o : o + nj],
                in0=src[:, :, i].unsqueeze(2).to_broadcast([P, COLS, nj]),
                in1=src[:, :, i + 1 : K1],
                op=op,
            )

    # --- full 5x5 gram matrix (for row sums) -> centering corrections
    GF = tile3(25, "GF")
    GF5 = GF[:].rearrange("p c (a b) -> p c a b", a=5)
    CP(GF[:, :, 0:25:6], N[:])                      # diagonal
    for i in range(K1 - 1):
        nj = K1 - 1 - i
        o = PAIR_OFF[i]
        CP(GF5[:, :, i, i + 1 : K1], G[:, :, o : o + nj])       # upper row i
        CP(GF5[:, :, i + 1 : K1, i], G[:, :, o : o + nj])       # lower col i
    M = tile3(K1, "M")                               # row sums = 5 * v_i . c
    nc.vector.tensor_reduce(out=M[:], in_=GF5, axis=X, op=Alu.add)
    Q = tile3(1, "Q")                                # sum of all = 25 * c . c
    nc.vector.tensor_reduce(out=Q[:, :, 0], in_=GF5, axis=XY, op=Alu.add)

    # centered pair dots / squared norms (only the cos path needs these)
    MP = tile3(NPAIR, "MP")
    pair_combine(MP, M, Alu.add)
    RC = tile3(NPAIR, "RC")
    STT(out=RC[:], in0=MP[:], scalar=-0.2, in1=G[:], op0=Alu.mult, op1=Alu.add)
    STT(
        out=RC[:],
        in0=Q[:].to_broadcast([P, COLS, NPAIR]),
        scalar=1.0 / 25.0,
        in1=RC[:],
        op0=Alu.mult,
        op1=Alu.add,
    )
    NC = tile3(K1, "NC")
    STT(out=NC[:], in0=M[:], scalar=-0.4, in1=N[:], op0=Alu.mult, op1=Alu.add)
    STT(
        out=NC[:],
        in0=Q[:].to_broadcast([P, COLS, K1]),
        scalar=1.0 / 25.0,
        in1=NC[:],
        op0=Alu.mult,
        op1=Alu.add,
    )

    RES = epool.tile([P, 3, COLS], f32, tag="RES", name="RES")

    # --- pairwise squared distances (raw values: centroid cancels)
    H = tile3(NPAIR, "H")
    pair_combine(H, N, Alu.add)
    D2 = tile3(NPAIR, "D2")
    STT(out=D2[:], in0=G[:], scalar=-2.0, in1=H[:], op0=Alu.mult, op1=Alu.add)
    E = tile3(NPAIR, "E")
    BIAS12 = epool.tile([P, 1], f32, tag="BIAS12", name="BIAS12")
    nc.vector.memset(BIAS12[:], 1e-12)
    nc.scalar.activation(E[:], D2[:], Act.Sqrt, bias=BIAS12[:])   # clips tiny negatives
    EMIN = tile3(1, "EMIN")
    EMAX = tile3(1, "EMAX")
    nc.vector.tensor_reduce(out=EMIN[:], in_=E[:], axis=X, op=Alu.min)
    nc.vector.tensor_reduce(out=EMAX[:], in_=E[:], axis=X, op=Alu.max)
    REMAX = tile3(1, "REMAX")
    nc.vector.reciprocal(REMAX[:], EMAX[:])
    TT(out=RES[:, 1, :], in0=EMIN[:, :, 0], in1=REMAX[:, :, 0], op=Alu.mult)

    # --- cos_ij = rho_c_ij / (n_i n_j); sum over the 10 pairs
    IN2 = tile3(K1, "IN2")
    nc.vector.reciprocal(IN2[:], NC[:])
    IP = tile3(NPAIR, "IP")
    pair_combine(IP, IN2, Alu.mult)
    nc.scalar.sqrt(IP[:], IP[:])
    COS = tile3(NPAIR, "COS")
    TT(out=COS[:], in0=RC[:], in1=IP[:], op=Alu.mult)
    nc.vector.tensor_reduce(out=RES[:, 0, :], in_=COS[:], axis=X, op=Alu.add)

    # --- Cayley-Menger degeneracy via det of the 4x4 gram of w_p = v_p - v_0
    # (raw values: w_p . w_q = G_pq - G_0p - G_0q + n2_0)
    Gd = tile3(4, "Gd")
    TT(
        out=Gd[:],
        in0=N[:, :, 1:K1],
        in1=N[:, :, 0].unsqueeze(2).to_broadcast([P, COLS, 4]),
        op=Alu.add,
    )
    STT(out=Gd[:], in0=G[:, :, 0:4], scalar=-2.0, in1=Gd[:], op0=Alu.mult, op1=Alu.add)
    Go = tile3(6, "Go")
    TT(
        out=Go[:],
        in0=G[:, :, 4:10],
        in1=N[:, :, 0].unsqueeze(2).to_broadcast([P, COLS, 6]),
        op=Alu.add,
    )
    # subtract G_0p (p-major: p=1 covers 3 pairs, p=2 covers 2, p=3 covers 1)
    for o, pidx, n in [(0, 0, 3), (3, 1, 2), (5, 2, 1)]:
        TT(
            out=Go[:, :, o : o + n],
            in0=Go[:, :, o : o + n],
            in1=G[:, :, pidx].unsqueeze(2).to_broadcast([P, COLS, n]),
            op=Alu.subtract,
        )
    # subtract G_0q (q lists are contiguous slices of G[:, :, 0:4])
    for o, q0, n in [(0, 1, 3), (3, 2, 2), (5, 3, 1)]:
        TT(
            out=Go[:, :, o : o + n],
            in0=Go[:, :, o : o + n],
            in1=G[:, :, q0 : q0 + n],
            op=Alu.subtract,
        )

    # det via Laplace on the first two rows: 12 2x2 minors
    # matrix [[A,B,C,D],[B,E,F,G],[C,F,H,I],[D,G,I,J]]
    A = Gd[:, :, 0]
    Ev = Gd[:, :, 1]
    Hv = Gd[:, :, 2]
    J = Gd[:, :, 3]
    Bv = Go[:, :, 0]
    Cv = Go[:, :, 1]
    Dv = Go[:, :, 2]
    F = Go[:, :, 3]
    G4 = Go[:, :, 4]
    I4 = Go[:, :, 5]
    svals = [(A, Ev, Bv, Bv), (A, F, Cv, Bv), (A, G4, Dv, Bv),
             (Bv, F, Cv, Ev), (Bv, G4, Dv, Ev), (Cv, G4, Dv, F)]
    cvals = [(Hv, J, I4, I4), (F, J, I4, G4), (F, I4, Hv, G4),
             (Cv, J, I4, Dv), (Cv, I4, Hv, Dv), (Cv, G4, F, Dv)]
    SV = tile3(6, "SV")
    CV = tile3(6, "CV")
    TMP = tile3(1, "TMP")
    for dst, vals in ((SV, svals), (CV, cvals)):
        for k, (x1, y1, x2, y2) in enumerate(vals):
            d = dst[:, :, k]
            TT(out=d, in0=x1, in1=y1, op=Alu.mult)
            TT(out=TMP[:, :, 0], in0=x2, in1=y2, op=Alu.mult)
            TT(out=d, in0=d, in1=TMP[:, :, 0], op=Alu.subtract)
    PV = tile3(6, "PV")
    TT(out=PV[:], in0=SV[:], in1=CV[:], op=Alu.mult)
    # det = p0 - p1 + p2 + p3 - p4 + p5
    DET = tile3(1, "DET")
    NEG = tile3(1, "NEG")
    TT(out=DET[:, :, 0], in0=PV[:, :, 0], in1=PV[:, :, 2], op=Alu.add)
    TT(out=DET[:, :, 0], in0=DET[:, :, 0], in1=PV[:, :, 3], op=Alu.add)
    TT(out=DET[:, :, 0], in0=DET[:, :, 0], in1=PV[:, :, 5], op=Alu.add)
    TT(out=NEG[:, :, 0], in0=PV[:, :, 1], in1=PV[:, :, 4], op=Alu.add)
    TT(out=DET[:, :, 0], in0=DET[:, :, 0], in1=NEG[:, :, 0], op=Alu.subtract)
    nc.vector.tensor_scalar(RES[:, 2, :], DET[:, :, 0], DEG_THRESH, None, op0=Alu.is_lt)

    # --- per-partition partial sums
    nc.vector.tensor_reduce(out=SUMS[:], in_=RES[:], axis=X, op=Alu.add)


def build():
    nc = bacc.Bacc(
        "TRN2",
        target_bir_lowering=False,
        debug=False,
        enable_asserts=False,
        num_devices=NCORES,
        num_swdge_queues=4,
    )
    pred = nc.dram_tensor("pred", [BPC * O, ROW], f32, kind="ExternalInput").ap()
    idx = nc.dram_tensor("idx", [P, NST, SUB * P // 16], i16, kind="ExternalInput").ap()
    out = nc.dram_tensor("out", [P, 3], f32, kind="ExternalOutput").ap()
    dbgG = nc.dram_tensor("dbgG", [P, COLS, NPAIR], f32, kind="ExternalOutput").ap() if DEBUG_DUMP else None
    dbgN = nc.dram_tensor("dbgN", [P, COLS, K1], f32, kind="ExternalOutput").ap() if DEBUG_DUMP else None

    with tile.TileContext(nc) as tc:
        with (
            tc.tile_pool(name="const", bufs=1) as cpool,
            tc.tile_pool(name="v", bufs=3) as vpool,
            tc.tile_pool(name="stat", bufs=1) as spool,
        ):
            idx_sb = cpool.tile([P, NST, SUB * P // 16], i16)
            nc.sync.dma_start(out=idx_sb[:], in_=idx)

            G = spool.tile([P, COLS, NPAIR], f32)
            N = spool.tile([P, COLS, K1], f32)

            for st in range(NST):
                cs = slice(st * SUB, (st + 1) * SUB)
                _emit_supertile(
                    nc,
                    vpool,
                    pred,
                    G[:, cs, :],
                    N[:, cs, :],
                    idx_sb[:, st],
                    st,
                )

            SUMS = spool.tile([P, 3], f32)
            _emit_epilogue(nc, spool, G[:], N[:], SUMS)
            nc.sync.dma_start(out=out, in_=SUMS[:])
            if DEBUG_DUMP:
                nc.sync.dma_start(out=dbgG, in_=G[:])
                nc.sync.dma_start(out=dbgN, in_=N[:])

    nc.compile()
    return nc


_NC = None


def _get_nc():
    global _NC
    if _NC is None:
        _NC = build()
    return _NC


def make_in_maps(predicted_simplices, sample_indices):
    pred = np.ascontiguousarray(predicted_simplices, dtype=np.float32)
    idx = np.ascontiguousarray(sample_indices, dtype=np.int32)
    in_maps = []
    for c in range(NCORES):
        p = pred[c * BPC : (c + 1) * BPC].reshape(BPC * O, ROW)
        # global (batch, sample) index -> local flat row id in this core's shard
        rowids = (
            idx[c * BPC : (c + 1) * BPC]
            + (np.arange(BPC, dtype=np.int32) * O)[:, None]
        ).reshape(SPC)
        # dma_gather index layout per call: flat position g lives at [g % 16, g // 16];
        # queue st is serviced by Q7 cores 2*st / 2*st+1 which read partitions
        # 32*st..+15 and +16..+31 (data replicated in both 16-partition blocks)
        ni = SUB * P
        ix = np.zeros((P, NST, ni // 16), np.int16)
        for st in range(NST):
            w = rowids[st * ni : (st + 1) * ni].astype(np.int16).reshape(ni // 16, 16).T
            ix[0:16, st] = w                        # CoreSim reads partitions 0-15
            ix[32 * st : 32 * st + 16, st] = w      # HW queue st: Q7 core 2*st
            ix[32 * st + 16 : 32 * st + 32, st] = w  # HW queue st: Q7 core 2*st+1
        in_maps.append(
            {"pred": np.ascontiguousarray(p), "idx": np.ascontiguousarray(ix)}
        )
    return in_maps


def combine(results):
    cs_total = 0.0
    reg_total = 0.0
    deg_total = 0.0
    for r in results:
        o = r["out"].astype(np.float64)
        cs_total += o[:, 0].sum()
        reg_total += o[:, 1].sum()
        deg_total += o[:, 2].sum()
    n = float(N_TOTAL)
    rose_loss = 0.5 - cs_total / (20.0 * n)
    quality_loss = 1.0 - reg_total / n
    volume_loss = deg_total / n
    total = 0.5 * rose_loss + 0.3 * quality_loss + 0.2 * volume_loss
    return np.float32(total)


def kernel(predicted_simplices, sample_indices):
    nc = _get_nc()
    in_maps = make_in_maps(predicted_simplices, sample_indices)
    res = run_bass_kernel_spmd(nc, in_maps, core_ids=list(range(NCORES)))
    return combine(res.results)
